# revision 1
# baseline (speedup 1.0000x reference)
"""BSplineKAN layer kernel for 8 Trainium2 NeuronCores.

Math
----
The reference computes, per element x = clip(x, -1, 1):
    y[n,o] = sum_{i,b} basis_b(x[n,i]) * coeff[o,i,b]  +  silu(x) @ w_base.T + bias
where basis is the 7-function clamped cubic B-spline basis on knots
{-1(x4), -0.5, 0, 0.5, 1(x4)}.  A quirk of the reference recurrence: at
x == 1.0 exactly (all clamped x >= 1 inputs) the basis row is all ZERO.

On [-1, 1) the basis functions are C^2 piecewise cubics with breakpoints at
+-0.5; we represent them exactly in a two-window local feature basis: for
each half H in {L: [-1,0), R: [0,1)} with center c_H = -+0.5, u = x - c_H,
window mask m_H, and knot-side mask g_H:
    feats_H = [m_H, m_H*u, m_H*u^2, m_H*u^3, g_H*u^3]
All ten features vanish at x == 1 (masks exclude it), reproducing the
reference's edge behavior exactly.  basis_b = M[f,b] @ feats (M integer/48,
exact).  M is folded into coeff on the host and silu/w_base appended as an
11th feature, giving one fused fp16 matmul
    y[n,o] = sum_{i,f} F_f(x[n,i]) * W[f,i,o] + bias
with K = 11*1024 = 11264.  Features are local (|u| <= 0.5), so the
contraction has no large-term cancellation; fp16 operands with fp32 PSUM
accumulation give ~5e-4 scale-relative absmax error (validated vs fp64).
Masks are exact in fp16 and the u-chain rounds at most 3 times, so the
all-fp16 feature pipeline adds no measurable error.

Distribution: 4-way batch x 2-way d_out mesh over 8 cores.  Per core:
x arrives host-transposed as (1024, 2048) fp32 (transposing on host is part
of sharding and keeps TensorE free of transposes), W-shard (11264, 512)
fp16 stays resident in SBUF, output (2048, 512) fp32.  Features are
computed on DVE (fp16 chain, 2x/4x modes) + ACT (affine/square/silu), and
TensorE runs back-to-back 88-tile K-accumulations into PSUM.
"""

import numpy as np

# ---- problem constants (hardcoded per contract) ----
N_FULL, D_IN, D_OUT = 8192, 1024, 1024
MESH_N, MESH_O = 4, 2                 # 4-way batch x 2-way d_out
N_SHARD = N_FULL // MESH_N            # 2048
O_SHARD = D_OUT // MESH_O             # 512
P = 128
NF = 11                               # 10 spline features + silu
IB = D_IN // P                        # 8 i-blocks
KT = IB * NF                          # 88 K-tiles
NCHUNK = 256                          # batch cols per pipeline chunk
NSUB = NCHUNK // P                    # 2
CHUNKS = N_SHARD // NCHUNK            # 8

# basis_b = sum_f feats_f * M[f, b];  feats order:
# [mL, mL*uL, mL*uL^2, mL*uL^3, gL*uL^3, mR, mR*uR, mR*uR^2, mR*uR^3, gR*uR^3]
_M48 = np.array([
    [0,    12,   28,   8,    0,    0,    0],
    [0,   -72,   24,   48,   0,    0,    0],
    [0,    144, -240,  96,   0,    0,    0],
    [-384, 672, -352,  64,   0,    0,    0],
    [384, -768,  576, -256,  64,   0,    0],
    [0,    0,    0,    8,    28,   12,   0],
    [0,    0,    0,   -48,  -24,   72,   0],
    [0,    0,    0,    96,  -240,  144,  0],
    [0,    0,   -64,   192, -224,  96,   0],
    [0,    0,    64,  -256,  576, -768,  384],
], dtype=np.float64)

_PROGRAM = None  # compiled Bass program, built once


def _build_program():
    import concourse.mybir as mybir
    import concourse.tile as tile
    from concourse import bacc

    f32 = mybir.dt.float32
    f16 = mybir.dt.float16
    Op = mybir.AluOpType
    Act = mybir.ActivationFunctionType

    nc = bacc.Bacc("TRN2", target_bir_lowering=False, debug=False)
    xt_d = nc.dram_tensor("xt", [D_IN, N_SHARD], f32, kind="ExternalInput").ap()
    w_d = nc.dram_tensor("wt", [KT * P, O_SHARD], f16, kind="ExternalInput").ap()
    b_d = nc.dram_tensor("biasb", [P, O_SHARD], f32, kind="ExternalInput").ap()
    y_d = nc.dram_tensor("y", [N_SHARD, O_SHARD], f32, kind="ExternalOutput").ap()

    with tile.TileContext(nc) as tc:
        with (
            tc.tile_pool(name="const", bufs=1) as const_pool,
            tc.tile_pool(name="wt", bufs=1) as wt_pool,
            tc.tile_pool(name="feat", bufs=2) as f_pool,
            tc.tile_pool(name="xc", bufs=2) as xc_pool,
            tc.tile_pool(name="tmp", bufs=2) as tmp_pool,
            tc.tile_pool(name="out", bufs=1) as out_pool,
            tc.tile_pool(name="pso", bufs=4, space="PSUM") as psum_out,
        ):
            bias_s = const_pool.tile([P, O_SHARD], f32)
            # tiny dummy activations up front so both ACT table sets load
            # concurrently with the initial DMAs instead of on the first
            # feature's critical path
            warm = const_pool.tile([P, 1], f32, name="warm")
            nc.gpsimd.memset(warm[:], 0.0)
            nc.scalar.activation(warm[:], warm[:], Act.Copy, bias=0.0)
            nc.scalar.activation(warm[:], warm[:], Act.Square)
            nc.scalar.activation(warm[:], warm[:], Act.Silu)
            b05 = const_pool.tile([P, 1], f32, name="b05")
            nc.gpsimd.memset(b05[:], 0.5)
            bm05 = const_pool.tile([P, 1], f32, name="bm05")
            nc.gpsimd.memset(bm05[:], -0.5)

            # warm-up: tiny matmuls on a zeroed tile fill the initial DMA
            # wait so the PE clock (HAM) is at full rate when the first real
            # matmul issues
            wz = const_pool.tile([P, P], f16, name="wz")
            nc.gpsimd.memset(wz[:], 0.0)
            pw = psum_out.tile([P, 64], f32, tag="pwarm", name="pwarm")
            for i in range(185):
                nc.tensor.matmul(pw[:], wz[:], wz[:, :64],
                                 start=(i == 0), stop=(i == 184))

            # startup DMA order minimizes time-to-first-real-matmul on the
            # serial queue: first half of chunk-0 x (enough for i-blocks 0-3),
            # then the first two weight slabs, then the rest of x, then the
            # remaining slabs.  One DMA per ib-slab of 11 weight tiles: HWDGE
            # charges per DMA instruction, so batching keeps the queue off
            # the critical path during the initial weight stream.
            xt_r = xt_d.rearrange("(ib p) n -> p ib n", p=P)
            xc0 = xc_pool.tile([P, IB, NCHUNK], f32, tag="xc", name="xc0")
            nc.sync.dma_start(xc0[:, :2], xt_r[:, :2, 0:NCHUNK])

            wt = {}
            def load_wt(ib):
                t = wt_pool.tile([P, NF, O_SHARD], f16, tag=f"wt_{ib}", name=f"wt_{ib}")
                r0 = ib * NF * P
                nc.sync.dma_start(
                    t[:], w_d[r0:r0 + NF * P, :].rearrange("(f p) o -> p f o", p=P))
                wt[ib] = t
            load_wt(0)
            load_wt(1)
            nc.sync.dma_start(xc0[:, 2:], xt_r[:, 2:, 0:NCHUNK])
            for ib in range(2, IB - 1):
                load_wt(ib)
            # chunk-1's x jumps ahead of the last weight slab (slab 7 is not
            # consumed until ~41us) so chunk-1 features start early
            xc1 = xc_pool.tile([P, IB, NCHUNK], f32, tag="xc", name="xc1")
            nc.sync.dma_start(xc1[:], xt_r[:, :, NCHUNK:2 * NCHUNK])
            load_wt(IB - 1)

            # bias is first needed at the first eviction (~30us in); loading
            # it after the weight slabs keeps it off the startup critical path
            nc.sync.dma_start(bias_s[:], b_d[:])

            for chunk in range(CHUNKS):
                c0 = chunk * NCHUNK
                # -- load x^T slices, clamp, build fp16 features --
                F = {}
                if chunk == 0:
                    # clamp in pieces matching the split chunk-0 DMAs so early
                    # i-blocks don't wait on later halves' arrival
                    xch = xc0
                    nc.vector.tensor_scalar(xch[:, :2], xch[:, :2],
                                            -1.0, 1.0, Op.max, Op.min)
                    nc.vector.tensor_scalar(xch[:, 2:], xch[:, 2:],
                                            -1.0, 1.0, Op.max, Op.min)
                else:
                    if chunk == 1:
                        xch = xc1
                    else:
                        xch = xc_pool.tile([P, IB, NCHUNK], f32, tag="xc", name="xc")
                        nc.sync.dma_start(xch[:], xt_r[:, :, c0:c0 + NCHUNK])
                    nc.vector.tensor_scalar(xch[:], xch[:], -1.0, 1.0, Op.max, Op.min)
                for ib in range(IB):
                    xcb = xch[:, ib]

                    def tmp(tag, w=2, bufs=2):
                        return tmp_pool.tile([P, w * NCHUNK], f16, tag=tag, name=tag, bufs=bufs)

                    def fpair(f):
                        # (128, 2*NCHUNK) tile holding K-tiles (ib, f) on the
                        # left half and (ib, f+5) on the right half
                        t = f_pool.tile([P, 2 * NCHUNK], f16, tag=f"F_{ib}_{f}",
                                        name=f"F_{ib}_{f}")
                        F[ib, f] = t
                        return t

                    N = NCHUNK
                    # cumulative masks on GpSimd (1-input ops run near line-rate there)
                    cB = tmp("cB", 1); nc.gpsimd.tensor_scalar(cB[:], xcb[:], 0.0, None, Op.is_ge)
                    cD = tmp("cD", 1, 1); nc.gpsimd.tensor_scalar(cD[:], xcb[:], 1.0, None, Op.is_ge)
                    # window masks (exact 0/1 in fp16): Fm = [mL | mR]
                    Fm = fpair(0)
                    nc.gpsimd.tensor_scalar(Fm[:, :N], xcb[:], 0.0, None, Op.is_lt)
                    nc.vector.tensor_tensor(Fm[:, N:], cB[:], cD[:], Op.subtract)
                    # u-chain: ACT writes both halves from the same xcb
                    U = tmp("U")
                    nc.scalar.activation(U[:, :N], xcb[:], Act.Copy, bias=0.5)
                    nc.scalar.activation(U[:, N:], xcb[:], Act.Copy, bias=-0.5)
                    U2 = tmp("U2")
                    nc.scalar.activation(U2[:, :N], xcb[:], Act.Square, bias=b05[:])
                    nc.scalar.activation(U2[:, N:], xcb[:], Act.Square, bias=bm05[:])
                    U3 = tmp("U3")
                    nc.vector.tensor_tensor(U3[:], U2[:], U[:], Op.mult)
                    # windowed monomials: one 512-wide fp16 op per L/R pair
                    nc.vector.tensor_tensor(fpair(1)[:], Fm[:], U[:], Op.mult)
                    nc.vector.tensor_tensor(fpair(2)[:], Fm[:], U2[:], Op.mult)
                    nc.vector.tensor_tensor(fpair(3)[:], Fm[:], U3[:], Op.mult)
                    # knot-side features: g_H * u^3 == m_H * relu(u^3), fused
                    nc.vector.scalar_tensor_tensor(fpair(4)[:], U3[:], 0.0, Fm[:],
                                                   Op.max, Op.mult)
                    fs = f_pool.tile([P, NCHUNK], f16, tag=f"F_{ib}_s", name=f"F_{ib}_s")
                    F[ib, 10] = fs
                    nc.scalar.activation(fs[:], xcb[:], Act.Silu)

                # -- matmuls. Chunk 0 runs k-major over both 128-batch
                # subtiles so each weight slab feeds two matmuls the moment its
                # DMA lands (PE stays ahead of the initial weight stream);
                # later chunks run the subtiles serially so group-0's eviction
                # overlaps group-1's matmuls --
                def lhs(ib, f, ns):
                    if f == 10:
                        return F[ib, 10][:, ns * P:(ns + 1) * P]
                    if f < 5:
                        return F[ib, f][:, ns * P:(ns + 1) * P]
                    return F[ib, f - 5][:, NCHUNK + ns * P:NCHUNK + (ns + 1) * P]

                def evict(ps, ns):
                    o = out_pool.tile([P, O_SHARD], f32, tag="out", name="outt")
                    nc.vector.tensor_tensor(o[:], ps[:], bias_s[:], Op.add)
                    r0 = c0 + ns * P
                    nc.sync.dma_start(y_d[r0:r0 + P, :], o[:])

                if chunk == 0:
                    pss = [psum_out.tile([P, O_SHARD], f32, tag=f"psout{ns}",
                                         name=f"psout{ns}", bufs=2)
                           for ns in range(NSUB)]
                    for k, (ib, f) in enumerate(
                            (ib, f) for ib in range(IB) for f in range(NF)):
                        for ns in range(NSUB):
                            nc.tensor.matmul(
                                pss[ns][:], lhs(ib, f, ns), wt[ib][:, f],
                                start=(k == 0), stop=(k == KT - 1))
                    for ns in range(NSUB):
                        evict(pss[ns], ns)
                else:
                    for ns in range(NSUB):
                        ps = psum_out.tile([P, O_SHARD], f32, tag=f"psout{ns}",
                                           name=f"psout{ns}", bufs=2)
                        for k, (ib, f) in enumerate(
                                (ib, f) for ib in range(IB) for f in range(NF)):
                            nc.tensor.matmul(
                                ps[:], lhs(ib, f, ns), wt[ib][:, f],
                                start=(k == 0), stop=(k == KT - 1))
                        evict(ps, ns)

    nc.compile()
    return nc


def _fold_weights(coeff, w_base):
    """Fold the feature->basis matrix into coeff; returns (K, D_OUT) fp16."""
    M = _M48 / 48.0
    c64 = np.asarray(coeff).astype(np.float64)
    # Wf[f, i, o] = sum_b M[f, b] * coeff[o, i, b]
    Wf = np.einsum('fb,oib->fio', M, c64)
    W11 = np.concatenate([Wf, np.asarray(w_base).astype(np.float64).T[None]], axis=0)  # (11, i, o)
    # pack K as (ib, f, p): row k = ib*(NF*P) + f*P + p  <->  W11[f, ib*P+p, o]
    Wt = W11.reshape(NF, IB, P, D_OUT).transpose(1, 0, 2, 3).reshape(KT * P, D_OUT)
    return Wt.astype(np.float16)


def kernel(x, coeff, w_base, bias):
    global _PROGRAM
    from concourse.bass_utils import run_bass_kernel_spmd

    if _PROGRAM is None:
        _PROGRAM = _build_program()
    nc = _PROGRAM

    x = np.asarray(x, dtype=np.float32)
    Wt = _fold_weights(coeff, w_base)
    bias = np.asarray(bias, dtype=np.float32)

    in_maps = []
    for core in range(8):
        cn, co = divmod(core, MESH_O)
        in_maps.append({
            "xt": np.ascontiguousarray(x[cn * N_SHARD:(cn + 1) * N_SHARD].T),
            "wt": np.ascontiguousarray(Wt[:, co * O_SHARD:(co + 1) * O_SHARD]),
            "biasb": np.ascontiguousarray(np.broadcast_to(
                bias[co * O_SHARD:(co + 1) * O_SHARD], (P, O_SHARD)).astype(np.float32)),
        })

    res = run_bass_kernel_spmd(nc, in_maps, list(range(8)))

    y = np.empty((N_FULL, D_OUT), dtype=np.float32)
    for core in range(8):
        cn, co = divmod(core, MESH_O)
        y[cn * N_SHARD:(cn + 1) * N_SHARD, co * O_SHARD:(co + 1) * O_SHARD] = \
            res.results[core]["y"]
    return y



# revision 4
# speedup vs baseline: 1.3536x; 1.3536x over previous
"""BSplineKAN layer kernel for 8 Trainium2 NeuronCores.

Math
----
Per element x (xc = clip(x, -1, 1)) the reference computes
    y[n,o] = sum_{i,b} basis_b(xc[n,i]) * coeff[o,i,b] + silu(xc) @ w_base.T + bias
with the 7-function clamped cubic B-spline basis on knots
{-1(x4), -0.5, 0, 0.5, 1(x4)}.  Reference quirk: at xc == 1.0 exactly the
basis row is all ZERO.

On [-1, 1) the basis lives in the 7-dim space spanned by the truncated-power
features of xc
    phi = [1, x, x^2, x^3, relu(-x-0.5)^3, relu(x)^3, relu(x-0.5)^3]
(exact integer/48 conversion matrix T, hardcoded below).  Two linear tricks
remove all masking from the device:
  * the constant feature's contribution sum_i W0[i,o] is added on the HOST
    (along with the input bias), so phi0 never hits the matmul;
  * the xc == 1 edge case becomes one extra linear feature e = (x >= 1)
    (computed from raw f32 x so fp16 rounding cannot flip the branch) whose
    weights cancel the limit value of the spline at 1^-:
    W_e = -(W0 + sum_f phi_f(1) * fp16(W_f)), folded against the fp16-rounded
    weights so the cancellation is exact at fp16 resolution.
Total contraction: 8 feature planes (x, x^2, x^3, 3 knot cubes, e, silu) *
1024 inputs = K 8192 (vs 11264 for the masked windowed basis) -- a 27%
matmul reduction at identical <1.5e-3 validated relative error.

All features are single DVE tensor_scalar chains (pow-op) or ACT table ops;
no tensor_tensor mask multiplies remain.  fp8/DoubleRow was evaluated and
rejected: any basis reachable with clamp/relu/pow ops has term cancellation
~5x, and e4m3's 1.6% grid accumulated over K=8192 measures 6-18e-2 relative
error (vs the 2e-2 gate); Dekker-style 2-3 pass repairs cost as much as fp16.

Distribution: 4-way batch x 2-way d_out mesh over 8 cores.  Per core:
x host-transposed (1024, 2048) f32, weights (8192, 512) f16 resident in
SBUF, output (2048, 512) f32; y += host bias after gather.
"""

import numpy as np

# ---- problem constants (hardcoded per contract) ----
N_FULL, D_IN, D_OUT = 8192, 1024, 1024
MESH_N, MESH_O = 4, 2                 # 4-way batch x 2-way d_out
N_SHARD = N_FULL // MESH_N            # 2048
O_SHARD = D_OUT // MESH_O             # 512
P = 128
NF = 8                                # feature planes entering the matmul
IB = D_IN // P                        # 8 i-blocks
KT = IB * NF                          # 64 K-tiles
NCHUNK = 256                          # batch cols per pipeline chunk
NSUB = NCHUNK // P                    # 2
CHUNKS = N_SHARD // NCHUNK            # 8

# basis_b(x) = sum_f T48[f, b]/48 * phi_f(x) on [-1, 1),
# phi = [1, x, x^2, x^3, relu(-x-0.5)^3, relu(x)^3, relu(x-0.5)^3]
_T48 = np.array([
    [0,    0,    8,    32,   8,    0,    0],
    [0,    0,   -48,   0,    48,   0,    0],
    [0,    0,    96,  -192,  96,   0,    0],
    [0,   -96,   224, -192,  64,   0,    0],
    [384, -768,  576, -256,  64,   0,    0],
    [0,    96,  -288,  384, -288,  96,   0],
    [0,    0,    64,  -256,  576, -768,  384],
], dtype=np.float64)

# matmul feature order (k-tile index f*IB + ib); phi index it maps to:
#   f0: xc        (phi1)      f4: relu(x^3)         (phi5)
#   f1: xc^2      (phi2)      f5: relu((x-.5)^3)    (phi6)
#   f2: xc^3      (phi3)      f6: e = (x >= 1)
#   f3: relu(-(x+.5)^3) (phi4)  f7: silu(xc)
_PHI_AT_1 = np.array([1.0, 1.0, 1.0, 0.0, 1.0, 0.125])  # phi_{1..6}(1)

_PROGRAM = None  # compiled Bass program, built once


def _build_program():
    import concourse.mybir as mybir
    import concourse.tile as tile
    from concourse import bacc

    f32 = mybir.dt.float32
    f16 = mybir.dt.float16
    Op = mybir.AluOpType
    Act = mybir.ActivationFunctionType

    nc = bacc.Bacc("TRN2", target_bir_lowering=False, debug=False)
    xt_d = nc.dram_tensor("xt", [D_IN, N_SHARD], f32, kind="ExternalInput").ap()
    w_d = nc.dram_tensor("wt", [KT * P, O_SHARD], f16, kind="ExternalInput").ap()
    y_d = nc.dram_tensor("y", [N_SHARD, O_SHARD], f32, kind="ExternalOutput").ap()

    with tile.TileContext(nc) as tc:
        with (
            tc.tile_pool(name="const", bufs=1) as const_pool,
            tc.tile_pool(name="wt", bufs=1) as wt_pool,
            tc.tile_pool(name="feat", bufs=2) as f_pool,
            tc.tile_pool(name="xc", bufs=2) as xc_pool,
            tc.tile_pool(name="tmp", bufs=2) as tmp_pool,
            tc.tile_pool(name="out", bufs=2) as out_pool,
            tc.tile_pool(name="pso", bufs=4, space="PSUM") as psum_out,
        ):
            # tiny dummy activations so both ACT table sets load during the
            # initial DMA wait instead of on the first feature's critical path
            warm = const_pool.tile([P, 1], f32, name="warm")
            nc.gpsimd.memset(warm[:], 0.0)
            nc.scalar.activation(warm[:], warm[:], Act.Square)
            nc.scalar.activation(warm[:], warm[:], Act.Silu)
            b05 = const_pool.tile([P, 1], f32, name="b05")
            nc.gpsimd.memset(b05[:], 0.5)
            bm05 = const_pool.tile([P, 1], f32, name="bm05")
            nc.gpsimd.memset(bm05[:], -0.5)

            # warm-up matmuls on a zeroed tile fill the initial DMA wait so
            # the PE clock (p-state ramp) is at full rate when the first real
            # matmul issues
            wz = const_pool.tile([P, P], f16, name="wz")
            nc.gpsimd.memset(wz[:], 0.0)
            pw = psum_out.tile([P, 64], f32, tag="pwarm", name="pwarm")
            for i in range(185):
                nc.tensor.matmul(pw[:], wz[:], wz[:, :64],
                                 start=(i == 0), stop=(i == 184))

            # startup DMA order: chunk-0 x first (features can start), then
            # weight slabs f-major matching matmul consumption order, with
            # chunk-1 x jumping the queue midway
            xt_r = xt_d.rearrange("(ib p) n -> p ib n", p=P)
            x0 = xc_pool.tile([P, IB, NCHUNK], f32, tag="xr", name="x0")
            nc.sync.dma_start(x0[:], xt_r[:, :, 0:NCHUNK])

            wt = {}
            def load_wt(f):
                t = wt_pool.tile([P, IB, O_SHARD], f16, tag=f"wt_{f}", name=f"wt_{f}")
                r0 = f * IB * P
                nc.sync.dma_start(
                    t[:], w_d[r0:r0 + IB * P, :].rearrange("(ib p) o -> p ib o", p=P))
                wt[f] = t
            for f in range(4):
                load_wt(f)
            x1 = xc_pool.tile([P, IB, NCHUNK], f32, tag="xr", name="x1")
            nc.sync.dma_start(x1[:], xt_r[:, :, NCHUNK:2 * NCHUNK])
            for f in range(4, NF):
                load_wt(f)

            for chunk in range(CHUNKS):
                c0 = chunk * NCHUNK
                W = IB * NCHUNK  # 2048-wide feature ops
                if chunk == 0:
                    xr = x0
                elif chunk == 1:
                    xr = x1
                else:
                    xr = xc_pool.tile([P, IB, NCHUNK], f32, tag="xr", name="xr")
                    nc.sync.dma_start(xr[:], xt_r[:, :, c0:c0 + NCHUNK])
                xrf = xr[:].rearrange("p ib n -> p (ib n)")

                def plane(name):
                    return f_pool.tile([P, W], f16, tag=f"F_{name}", name=f"F_{name}")

                F = [None] * NF
                # f6: edge feature from RAW f32 x (fp16 rounding must not
                # flip the x==1 branch); Pool is otherwise idle
                F[6] = plane("e")
                nc.gpsimd.tensor_scalar(F[6][:], xrf, 1.0, None, Op.is_ge)
                # f0: xc = clip(x, -1, 1) -> f16
                F[0] = plane("xc")
                nc.vector.tensor_scalar(F[0][:], xrf, -1.0, 1.0, Op.max, Op.min)
                xc = F[0][:]
                # f1: xc^2 (ACT), f7: silu(xc) (ACT)
                F[1] = plane("x2")
                nc.scalar.activation(F[1][:], xc, Act.Square)
                F[7] = plane("sil")
                nc.scalar.activation(F[7][:], xc, Act.Silu)
                # f2: xc^3 = xc^2 * xc; f4: relu(xc^3)
                F[2] = plane("x3")
                nc.vector.tensor_tensor(F[2][:], F[1][:], xc, Op.mult)
                F[4] = plane("r0")
                nc.vector.tensor_scalar(F[4][:], F[2][:], 0.0, None, Op.max)
                # f3: relu(-x-.5)^3 = relu(w)*w^2, w = -x-.5 (w^2 = (x+.5)^2)
                w2 = tmp_pool.tile([P, W], f16, tag="w2", name="w2")
                nc.scalar.activation(w2[:], xc, Act.Square, bias=b05[:])
                wn = tmp_pool.tile([P, W], f16, tag="wn", name="wn")
                nc.vector.tensor_scalar(wn[:], xc, -1.0, 0.5, Op.mult, Op.subtract)
                F[3] = plane("kL")
                nc.vector.scalar_tensor_tensor(F[3][:], wn[:], 0.0, w2[:],
                                               Op.max, Op.mult)
                # f5: relu(x-.5)^3 = relu(v)*v^2, v = x-.5
                v2 = tmp_pool.tile([P, W], f16, tag="v2", name="v2")
                nc.scalar.activation(v2[:], xc, Act.Square, bias=bm05[:])
                vn = tmp_pool.tile([P, W], f16, tag="vn", name="vn")
                nc.vector.tensor_scalar(vn[:], xc, -0.5, None, Op.add)
                F[5] = plane("kR")
                nc.vector.scalar_tensor_tensor(F[5][:], vn[:], 0.0, v2[:],
                                               Op.max, Op.mult)

                # -- matmuls. Chunk 0 runs k-major over both 128-batch
                # subtiles so each weight slab feeds two matmuls as its DMA
                # lands; later chunks run subtiles serially so group-0's
                # eviction overlaps group-1's matmuls --
                def lhs(f, ib, ns):
                    o = ib * NCHUNK + ns * P
                    return F[f][:, o:o + P]

                def evict(ps, ns):
                    o = out_pool.tile([P, O_SHARD], f32, tag="out", name="outt")
                    nc.vector.tensor_copy(o[:], ps[:])
                    r0 = c0 + ns * P
                    nc.sync.dma_start(y_d[r0:r0 + P, :], o[:])

                if chunk == 0:
                    pss = [psum_out.tile([P, O_SHARD], f32, tag=f"psout{ns}",
                                         name=f"psout{ns}", bufs=2)
                           for ns in range(NSUB)]
                    for k in range(KT):
                        f, ib = divmod(k, IB)
                        for ns in range(NSUB):
                            nc.tensor.matmul(
                                pss[ns][:], lhs(f, ib, ns), wt[f][:, ib],
                                start=(k == 0), stop=(k == KT - 1))
                    for ns in range(NSUB):
                        evict(pss[ns], ns)
                else:
                    for ns in range(NSUB):
                        ps = psum_out.tile([P, O_SHARD], f32, tag=f"psout{ns}",
                                           name=f"psout{ns}", bufs=2)
                        for k in range(KT):
                            f, ib = divmod(k, IB)
                            nc.tensor.matmul(
                                ps[:], lhs(f, ib, ns), wt[f][:, ib],
                                start=(k == 0), stop=(k == KT - 1))
                        evict(ps, ns)

    nc.compile()
    return nc


def _fold_weights(coeff, w_base):
    """Returns (Wt fp16 (KT*P, D_OUT) packed k=(f,ib,p), host_bias fp64 (D_OUT,))."""
    T = _T48 / 48.0
    c64 = np.asarray(coeff).astype(np.float64)
    Wf = np.einsum('fb,oib->fio', T, c64)          # (7, i, o); Wf[0] = const
    W16 = [Wf[f].astype(np.float16) for f in range(1, 7)]
    # edge feature: cancels the 1^- limit against the fp16-rounded weights
    We = -(Wf[0] + np.einsum('f,fio->io', _PHI_AT_1,
                             np.stack([w.astype(np.float64) for w in W16])))
    planes = W16 + [We.astype(np.float16),
                    np.asarray(w_base).astype(np.float16).T]   # 8 planes (i, o)
    # pack k = (f, ib, p) <-> plane[f][ib*P+p, o]
    Wt = np.stack([p.astype(np.float16) for p in planes])      # (8, i, o)
    Wt = Wt.reshape(NF, IB, P, D_OUT).reshape(KT * P, D_OUT)
    host_bias = Wf[0].sum(axis=0)                              # (o,)
    return Wt, host_bias


def kernel(x, coeff, w_base, bias):
    global _PROGRAM
    from concourse.bass_utils import run_bass_kernel_spmd

    if _PROGRAM is None:
        _PROGRAM = _build_program()
    nc = _PROGRAM

    x = np.asarray(x, dtype=np.float32)
    Wt, host_bias = _fold_weights(coeff, w_base)
    badd = (host_bias + np.asarray(bias).astype(np.float64)).astype(np.float32)

    in_maps = []
    for core in range(8):
        cn, co = divmod(core, MESH_O)
        in_maps.append({
            "xt": np.ascontiguousarray(x[cn * N_SHARD:(cn + 1) * N_SHARD].T),
            "wt": np.ascontiguousarray(Wt[:, co * O_SHARD:(co + 1) * O_SHARD]),
        })

    res = run_bass_kernel_spmd(nc, in_maps, list(range(8)))

    y = np.empty((N_FULL, D_OUT), dtype=np.float32)
    for core in range(8):
        cn, co = divmod(core, MESH_O)
        y[cn * N_SHARD:(cn + 1) * N_SHARD, co * O_SHARD:(co + 1) * O_SHARD] = \
            res.results[core]["y"]
    y += badd[None, :]
    return y


# revision 20
# speedup vs baseline: 1.5624x; 1.1543x over previous
"""BSplineKAN layer kernel for 8 Trainium2 NeuronCores.

Math
----
Per element x (xc = clip(x, -1, 1)) the reference computes
    y[n,o] = sum_{i,b} basis_b(xc[n,i]) * coeff[o,i,b] + silu(xc) @ w_base.T + bias
with the 7-function clamped cubic B-spline basis on knots
{-1(x4), -0.5, 0, 0.5, 1(x4)}.  Reference quirk: at xc == 1.0 exactly the
basis row is all ZERO.

On [-1, 1) the basis lives in the 7-dim space spanned by the truncated-power
features of xc
    phi = [1, x, x^2, x^3, relu(-x-0.5)^3, relu(x)^3, relu(x-0.5)^3]
(exact integer/48 conversion matrix T, hardcoded below).  Linear tricks
remove all masking from the device:
  * the constant feature's contribution sum_i W0[i,o] is added on the HOST
    (with the input bias), so phi0 never hits the matmul;
  * the xc == 1 edge case becomes one extra linear feature e = (x >= 1)
    whose weights cancel the spline's 1^- limit:
    W_e = -(W0 + sum_f phi_f(1) * fp16(W_f)), folded against the fp16-rounded
    weights so the cancellation is exact at working resolution.  The edge
    decision is made on the HOST in f32 (f16 rounding must not flip the
    branch) and shipped as a 2.0 sentinel inside the f16 x tensor.

Contraction layout (K = 8 * 1024):
  * 6 truncated-power features run as 48 fp16 K-tiles (their weights carry
    ~5x term cancellation, so fp8 anywhere on them measures 3-25e-2 error
    vs the 2e-2 gate -- evaluated and rejected);
  * e and silu run as 16 fp8e4 DoubleRow pairs (0.5 cycles/row) carrying
    Dekker-split weights: pair = (F, F/16) x (q8(W), q8(16*(W - q8(W)))),
    giving ~fp16 weight precision at half the PE cost.  e is 0/1 (fp8
    exact); silu's fp8 feature rounding adds ~2e-3 relative error.
  PE time: 48*512 + 16*256 cycles per 128-batch group = 191us/core vs
  300us for the 11-feature fp16 baseline.

Distribution: 4-way batch x 2-way d_out mesh over 8 cores.  Per core:
x host-encoded (1024, 2048) f16, weights (6144, 512) f16 + (4096, 512) f8
resident in SBUF, output (2048, 512) f32; y += host bias after gather.
"""

import numpy as np

# ---- problem constants (hardcoded per contract) ----
N_FULL, D_IN, D_OUT = 8192, 1024, 1024
MESH_N, MESH_O = 4, 2                 # 4-way batch x 2-way d_out
N_SHARD = N_FULL // MESH_N            # 2048
O_SHARD = D_OUT // MESH_O             # 512
P = 128
NF16 = 6                              # fp16 feature planes (phi_1..phi_6)
IB = D_IN // P                        # 8 i-blocks
KT16 = NF16 * IB                      # 48 fp16 K-tiles
NPAIR = 2 * IB                        # 16 fp8 DoubleRow pairs (e, silu)
NCHUNK = 256                          # batch cols per pipeline chunk
WARMUP = 104                          # PE clock-ramp filler matmuls
NSUB = NCHUNK // P                    # 2
CHUNKS = N_SHARD // NCHUNK            # 8

# basis_b(x) = sum_f T48[f, b]/48 * phi_f(x) on [-1, 1),
# phi = [1, x, x^2, x^3, relu(-x-0.5)^3, relu(x)^3, relu(x-0.5)^3]
_T48 = np.array([
    [0,    0,    8,    32,   8,    0,    0],
    [0,    0,   -48,   0,    48,   0,    0],
    [0,    0,    96,  -192,  96,   0,    0],
    [0,   -96,   224, -192,  64,   0,    0],
    [384, -768,  576, -256,  64,   0,    0],
    [0,    96,  -288,  384, -288,  96,   0],
    [0,    0,    64,  -256,  576, -768,  384],
], dtype=np.float64)

# fp16 plane order (K-tile index f*IB + ib); phi index it maps to:
#   f0: xc (phi1)   f1: xc^2 (phi2)        f2: xc^3 (phi3)
#   f3: relu(-(x+.5)^3) (phi4)  f4: relu(x^3) (phi5)  f5: relu((x-.5)^3) (phi6)
_PHI_AT_1 = np.array([1.0, 1.0, 1.0, 0.0, 1.0, 0.125])  # phi_{1..6}(1)

_PROGRAM = None  # compiled Bass program, built once


def _build_program():
    import concourse.mybir as mybir
    import concourse.tile as tile
    from concourse import bacc

    f32 = mybir.dt.float32
    f16 = mybir.dt.float16
    f8 = mybir.dt.float8e4
    Op = mybir.AluOpType
    Act = mybir.ActivationFunctionType
    DR = mybir.MatmulPerfMode.DoubleRow

    nc = bacc.Bacc("TRN2", target_bir_lowering=False, debug=False)
    xt_d = nc.dram_tensor("xt", [D_IN, N_SHARD], f16, kind="ExternalInput").ap()
    w_d = nc.dram_tensor("wt", [KT16 * P, O_SHARD], f16, kind="ExternalInput").ap()
    w8_d = nc.dram_tensor("wt8", [2 * NPAIR * P, O_SHARD], f8,
                          kind="ExternalInput").ap()
    y_d = nc.dram_tensor("y", [N_SHARD, O_SHARD], f32, kind="ExternalOutput").ap()

    with tile.TileContext(nc) as tc:
        with (
            tc.tile_pool(name="const", bufs=1) as const_pool,
            tc.tile_pool(name="wt", bufs=1) as wt_pool,
            tc.tile_pool(name="feat", bufs=2) as f_pool,
            tc.tile_pool(name="xc", bufs=2) as xc_pool,
            tc.tile_pool(name="tmp", bufs=2) as tmp_pool,
            tc.tile_pool(name="out", bufs=2) as out_pool,
            tc.tile_pool(name="pso", bufs=4, space="PSUM") as psum_out,
        ):
            # warm-up tile memset FIRST so PE warm-up matmuls start asap and
            # anchor the p-state clock ramp; they fill until the first weight
            # slab + x chunk land (~5.3us)
            wz = const_pool.tile([P, P], f16, name="wz")
            nc.gpsimd.memset(wz[:], 0.0)
            pw = psum_out.tile([P, 64], f32, tag="pwarm", name="pwarm", bufs=1)
            for i in range(WARMUP):
                nc.tensor.matmul(pw[:], wz[:], wz[:, :64],
                                 start=(i == 0), stop=(i == WARMUP - 1))

            # tiny dummy activations so both ACT table sets load during the
            # initial DMA wait instead of on the first feature's critical path
            warm = const_pool.tile([P, 1], f32, name="warm")
            nc.gpsimd.memset(warm[:], 0.0)
            nc.scalar.activation(warm[:], warm[:], Act.Square)
            nc.scalar.activation(warm[:], warm[:], Act.Silu)
            b05 = const_pool.tile([P, 1], f32, name="b05")
            nc.gpsimd.memset(b05[:], 0.5)
            bm05 = const_pool.tile([P, 1], f32, name="bm05")
            nc.gpsimd.memset(bm05[:], -0.5)

            # startup DMA order (one serialized DMA device): first x chunk,
            # then fp16 weight slabs in k-consumption order (slab 0 in
            # quarters, rest in halves so supply granularity keeps the PE
            # fed from first-matmul t~5.3us), chunk-1 x, then the fp8 pair
            # slabs consumed at the end of each chunk's K sweep
            xt_r = xt_d.rearrange("(ib p) n -> p ib n", p=P)
            w_r = w_d.rearrange("(f ib p) o -> p f ib o", p=P, f=NF16)
            w8_r = w8_d.rearrange("(pi two p) o -> p pi two o", p=P, two=2)
            x0 = xc_pool.tile([P, IB, NCHUNK], f16, tag="xr", name="x0")
            wt = wt_pool.tile([P, NF16, IB, O_SHARD], f16, name="wt")
            wt8 = wt_pool.tile([P, NPAIR, 2, O_SHARD], f8, name="wt8")
            nc.sync.dma_start(x0[:], xt_r[:, :, 0:NCHUNK])
            for q in range(4):
                nc.sync.dma_start(wt[:, 0, 2 * q:2 * q + 2], w_r[:, 0, 2 * q:2 * q + 2])
            H2 = IB // 2
            for f in range(1, NF16):
                nc.sync.dma_start(wt[:, f, :H2], w_r[:, f, :H2])
                nc.sync.dma_start(wt[:, f, H2:], w_r[:, f, H2:])
            x1 = xc_pool.tile([P, IB, NCHUNK], f16, tag="xr", name="x1")
            nc.sync.dma_start(x1[:], xt_r[:, :, NCHUNK:2 * NCHUNK])
            for h in range(4):
                nc.sync.dma_start(wt8[:, 4 * h:4 * h + 4], w8_r[:, 4 * h:4 * h + 4])

            for chunk in range(CHUNKS):
                c0 = chunk * NCHUNK
                W = IB * NCHUNK  # 2048-wide feature ops
                if chunk == 0:
                    xr = x0
                elif chunk == 1:
                    xr = x1
                else:
                    xr = xc_pool.tile([P, IB, NCHUNK], f16, tag="xr", name="xr")
                    nc.sync.dma_start(xr[:], xt_r[:, :, c0:c0 + NCHUNK])
                xrf = xr[:].rearrange("p ib n -> p (ib n)")

                def plane(name):
                    return f_pool.tile([P, W], f16, tag=f"F_{name}", name=f"F_{name}")

                F = [None] * NF16
                # f0: xc = clip(x', -1, 1) -> f16 (also squashes the sentinel)
                F[0] = plane("xc")
                nc.vector.tensor_scalar(F[0][:], xrf, -1.0, 1.0, Op.max, Op.min)
                xc = F[0][:]
                # fp8 pair planes: e = (x' >= 1.5) decodes the edge sentinel
                # (exact 0/1 in fp8); the second subplane is F/16 to apply
                # the Dekker low weight's 1/16 scale on the feature side
                Fe = f_pool.tile([P, 2, W], f8, tag="Fe", name="Fe")
                nc.gpsimd.tensor_scalar(Fe[:, 0], xrf, 1.5, None, Op.is_ge)
                nc.vector.tensor_scalar(Fe[:, 1], xrf, 1.5, 0.0625, Op.is_ge, Op.mult)
                Fs = f_pool.tile([P, 2, W], f8, tag="Fs", name="Fs")
                nc.scalar.activation(Fs[:, 0], xc, Act.Silu)
                nc.vector.tensor_scalar(Fs[:, 1], Fs[:, 0], 0.0625, None, Op.mult)
                # f1: xc^2 (ACT)
                F[1] = plane("x2")
                nc.scalar.activation(F[1][:], xc, Act.Square)
                # f2: xc^3 = xc^2 * xc; f4: relu(xc^3)
                F[2] = plane("x3")
                nc.vector.tensor_tensor(F[2][:], F[1][:], xc, Op.mult)
                F[4] = plane("r0")
                nc.vector.tensor_scalar(F[4][:], F[2][:], 0.0, None, Op.max)
                # f3: relu(-x-.5)^3 = relu(w)*w^2, w = -x-.5 (w^2 = (x+.5)^2)
                w2 = tmp_pool.tile([P, W], f16, tag="w2", name="w2")
                nc.scalar.activation(w2[:], xc, Act.Square, bias=b05[:])
                wn = tmp_pool.tile([P, W], f16, tag="wn", name="wn")
                nc.vector.tensor_scalar(wn[:], xc, -1.0, 0.5, Op.mult, Op.subtract)
                F[3] = plane("kL")
                nc.vector.scalar_tensor_tensor(F[3][:], wn[:], 0.0, w2[:],
                                               Op.max, Op.mult)
                # f5: relu(x-.5)^3 = relu(v)*v^2, v = x-.5
                v2 = tmp_pool.tile([P, W], f16, tag="v2", name="v2")
                nc.scalar.activation(v2[:], xc, Act.Square, bias=bm05[:])
                vn = tmp_pool.tile([P, W], f16, tag="vn", name="vn")
                nc.vector.tensor_scalar(vn[:], xc, -0.5, None, Op.add)
                F[5] = plane("kR")
                nc.vector.scalar_tensor_tensor(F[5][:], vn[:], 0.0, v2[:],
                                               Op.max, Op.mult)

                # -- matmuls: 48 fp16 K-tiles then 16 fp8 DoubleRow pairs in
                # one PSUM accumulation group.  Chunk 0 runs k-major over
                # both 128-batch subtiles so each weight slab feeds two
                # matmuls as its DMA lands; later chunks run subtiles
                # serially so one group's eviction overlaps the next's
                # matmuls --
                def sweep(ps, ns, o0, o1):
                    for k in range(KT16):
                        f, ib = divmod(k, IB)
                        off = ib * NCHUNK + ns * P
                        nc.tensor.matmul(
                            ps, F[f][:, off:off + P], wt[:, f, ib, o0:o1],
                            start=(k == 0), stop=False)
                    for pi in range(NPAIR):
                        Fp = Fe if pi < IB else Fs
                        off = (pi % IB) * NCHUNK + ns * P
                        nc.tensor.matmul(
                            ps, Fp[:, :, off:off + P], wt8[:, pi, :, o0:o1],
                            start=False, stop=(pi == NPAIR - 1), perf_mode=DR)

                def evict(ps, ns):
                    o = out_pool.tile([P, O_SHARD], f32, tag="out", name="outt")
                    nc.vector.tensor_copy(o[:], ps[:])
                    r0 = c0 + ns * P
                    nc.sync.dma_start(y_d[r0:r0 + P, :], o[:])

                if chunk == 0:
                    pss = [psum_out.tile([P, O_SHARD], f32, tag=f"psout{ns}",
                                         name=f"psout{ns}", bufs=2)
                           for ns in range(NSUB)]
                    for k in range(KT16):
                        f, ib = divmod(k, IB)
                        for ns in range(NSUB):
                            off = ib * NCHUNK + ns * P
                            nc.tensor.matmul(
                                pss[ns][:], F[f][:, off:off + P], wt[:, f, ib],
                                start=(k == 0), stop=False)
                    for pi in range(NPAIR):
                        Fp = Fe if pi < IB else Fs
                        for ns in range(NSUB):
                            off = (pi % IB) * NCHUNK + ns * P
                            nc.tensor.matmul(
                                pss[ns][:], Fp[:, :, off:off + P], wt8[:, pi],
                                start=False, stop=(pi == NPAIR - 1), perf_mode=DR)
                    for ns in range(NSUB):
                        evict(pss[ns], ns)
                elif chunk < CHUNKS - 1:
                    for ns in range(NSUB):
                        ps = psum_out.tile([P, O_SHARD], f32, tag=f"psout{ns}",
                                           name=f"psout{ns}", bufs=2)
                        sweep(ps[:], ns, 0, O_SHARD)
                        evict(ps, ns)
                else:
                    # last chunk: subtile 1 accumulates in shrinking column
                    # pieces so only the final small piece's eviction chain
                    # is exposed after the very last matmul
                    ps = psum_out.tile([P, O_SHARD], f32, tag="psout0",
                                       name="psout0", bufs=2)
                    sweep(ps[:], 0, 0, O_SHARD)
                    evict(ps, 0)
                    PIECES = (256, 128, 64, 64)
                    off = 0
                    for h, HW in enumerate(PIECES):
                        psh = psum_out.tile([P, HW], f32, tag=f"psl{h % 2}",
                                            name=f"psl{h % 2}", bufs=1)[:]
                        sweep(psh, 1, off, off + HW)
                        o = out_pool.tile([P, HW], f32, tag=f"outl{h}",
                                          name=f"outl{h}")
                        nc.vector.tensor_copy(o[:], psh)
                        r0 = c0 + P
                        nc.sync.dma_start(y_d[r0:r0 + P, off:off + HW], o[:])
                        off += HW

    nc.compile()
    return nc


def _fold_weights(coeff, w_base):
    """Returns (Wt16 (KT16*P, D_OUT) f16, W8 (2*NPAIR*P, D_OUT) f8,
    host_bias (D_OUT,) f64)."""
    import ml_dtypes
    f8t = ml_dtypes.float8_e4m3

    T = _T48 / 48.0
    c64 = np.asarray(coeff).astype(np.float64)
    Wf = np.einsum('fb,oib->fio', T, c64)          # (7, i, o); Wf[0] = const
    W16 = [Wf[f].astype(np.float16) for f in range(1, 7)]
    Wt16 = np.stack(W16)                           # (6, i, o)
    Wt16 = Wt16.reshape(NF16, IB, P, D_OUT).reshape(KT16 * P, D_OUT)

    # edge feature: cancels the 1^- limit against the fp16-rounded weights
    We = -(Wf[0] + np.einsum('f,fio->io', _PHI_AT_1,
                             np.stack([w.astype(np.float64) for w in W16])))
    Ws = np.asarray(w_base).astype(np.float64).T   # (i, o)

    def dekker8(Wx):
        hi = Wx.astype(f8t)
        lo = ((Wx - hi.astype(np.float64)) * 16.0).astype(f8t)
        return hi, lo

    We_hi, We_lo = dekker8(We)
    Ws_hi, Ws_lo = dekker8(Ws)
    # pack pair-major: pi 0..7 = e per i-block, pi 8..15 = silu per i-block;
    # row k = (pi*2 + two)*P + p
    W8 = np.zeros((2 * NPAIR * P, D_OUT), dtype=f8t)
    for ib in range(IB):
        rows = slice(ib * P, (ib + 1) * P)
        W8[(2 * ib) * P:(2 * ib + 1) * P] = We_hi[rows]
        W8[(2 * ib + 1) * P:(2 * ib + 2) * P] = We_lo[rows]
        pj = 2 * (IB + ib)
        W8[pj * P:(pj + 1) * P] = Ws_hi[rows]
        W8[(pj + 1) * P:(pj + 2) * P] = Ws_lo[rows]

    host_bias = Wf[0].sum(axis=0)                  # (o,)
    return Wt16, W8, host_bias


def kernel(x, coeff, w_base, bias):
    global _PROGRAM
    from concourse.bass_utils import run_bass_kernel_spmd

    if _PROGRAM is None:
        _PROGRAM = _build_program()
    nc = _PROGRAM

    x = np.asarray(x, dtype=np.float32)
    # sentinel-encode the x>=1 edge cases as 2.0, then f16 (see _build_program)
    xs = np.where(x >= 1.0, np.float32(2.0),
                  np.clip(x, -1.0, 1.0)).astype(np.float16)
    Wt16, W8, host_bias = _fold_weights(coeff, w_base)
    badd = (host_bias + np.asarray(bias).astype(np.float64)).astype(np.float32)

    in_maps = []
    for core in range(8):
        cn, co = divmod(core, MESH_O)
        osl = slice(co * O_SHARD, (co + 1) * O_SHARD)
        in_maps.append({
            "xt": np.ascontiguousarray(xs[cn * N_SHARD:(cn + 1) * N_SHARD].T),
            "wt": np.ascontiguousarray(Wt16[:, osl]),
            "wt8": np.ascontiguousarray(W8[:, osl]),
        })

    res = run_bass_kernel_spmd(nc, in_maps, list(range(8)))

    y = np.empty((N_FULL, D_OUT), dtype=np.float32)
    for core in range(8):
        cn, co = divmod(core, MESH_O)
        y[cn * N_SHARD:(cn + 1) * N_SHARD, co * O_SHARD:(co + 1) * O_SHARD] = \
            res.results[core]["y"]
    y += badd[None, :]
    return y
